# revision 16
# baseline (speedup 1.0000x reference)
"""Causal self-attention (B=1, T=4096, C=1024, H=16) on 8 TRN2 NeuronCores.

Sharding: head/tensor parallel. Core c owns heads (2c, 2c+1):
  - Wq/Wk/Wv column-split by head -> each core computes its 128 q/k/v features.
  - Attention is local per head (no cross-core comm).
  - Wp row-split: each core computes a partial (T, C) output; host sums the 8
    partials and adds bp.

Device layouts (per core):
  xT   (C=1024, T=4096)  x transposed (replicated), contraction dim on partitions
  qT/kT (128, T)         2 heads stacked on partitions (64 each)
  v    (T-chunks, 65)    natural layout + ones column for softmax row-sums
  scores computed transposed: S^T[s, q] = k @ q^T per head, so
    - exp on ScalarE (scale=1/sqrt(D) fused)
    - sums via the v ones-column during the AV matmul
    - AV output y^T (d, q) is directly the stationary operand for the Wp matmul
"""

import numpy as np

import concourse.bass as bass
import concourse.bacc as bacc
import concourse.mybir as mybir
import concourse.tile as tile
from concourse.bass_utils import run_bass_kernel_spmd

T = 4096
C = 1024
H = 16
D = 64
N_CORES = 8
F = 128          # features per core (2 heads x 64)
TB = 512         # token block for QKV/proj phases
NTB = T // TB    # 8
SC = 128         # key/s chunk
NSC = T // SC    # 32
QT = 512         # q tile in attention phase
NQT = T // QT    # 8

FP32 = mybir.dt.float32
F32R = mybir.dt.float32r
BF16 = mybir.dt.bfloat16

# "f32r"  : all matmuls fp32r (safest)
# "mix"   : q/k stored bf16 -> scores matmul at full rate; value path fp32r
# "bf16"  : whole attention path bf16 (fastest, loosest)
PRECISION = "f32r"

SOFTMAX_SCALE = 1.0 / float(np.sqrt(D))

# Set by test harness to enable NTFF profiling; results stashed in last_results.
TRACE = False
TRACE_KW = {}
last_results = None

_nc_cache = None


def round_fp32r(a):
    """Round fp32 array to the fp32r grid (RNE at 12 low mantissa bits),
    matching the PE's fp32r operand format."""
    u = np.ascontiguousarray(a, np.float32).view(np.uint32).astype(np.uint64)
    low = u & 0xFFF
    base = u & 0xFFFFF000
    rnd = (low > 0x800) | ((low == 0x800) & (((u >> 12) & 1) == 1))
    base = base + rnd.astype(np.uint64) * 0x1000
    return (base & 0xFFFFFFFF).astype(np.uint32).view(np.float32)


def _build(nc):
    ATT = BF16 if PRECISION == "bf16" else F32R
    xT = nc.dram_tensor("xT", [C, T], ATT, kind="ExternalInput")
    wqT = nc.dram_tensor("wqT", [C, F], ATT, kind="ExternalInput")
    wkT = nc.dram_tensor("wkT", [C, F], ATT, kind="ExternalInput")
    wvT = nc.dram_tensor("wvT", [C, F], ATT, kind="ExternalInput")
    bq = nc.dram_tensor("bq", [F, 1], FP32, kind="ExternalInput")
    bk = nc.dram_tensor("bk", [F, 1], FP32, kind="ExternalInput")
    bv = nc.dram_tensor("bv", [1, F], FP32, kind="ExternalInput")
    wpT = nc.dram_tensor("wpT", [F, C], F32R, kind="ExternalInput")
    mask = nc.dram_tensor("mask", [SC, SC], ATT, kind="ExternalInput")
    ident = nc.dram_tensor("ident", [128, 128], FP32, kind="ExternalInput")
    out_p = nc.dram_tensor("out_p", [T, C], FP32, kind="ExternalOutput")

    with tile.TileContext(nc) as tc:
        _body(tc, nc, xT, wqT, wkT, wvT, bq, bk, bv, wpT, mask, ident, out_p)
    return nc


def _body(tc, nc, xT, wqT, wkT, wvT, bq, bk, bv, wpT, mask, ident, out_p):
    ATT = BF16 if PRECISION == "bf16" else F32R
    QK = BF16 if PRECISION in ("bf16", "mix") else F32R
    from contextlib import ExitStack
    with ExitStack() as ctx:
        consts = ctx.enter_context(tc.tile_pool(name="consts", bufs=1))
        persist = ctx.enter_context(tc.tile_pool(name="persist", bufs=1))
        vpool = ctx.enter_context(tc.tile_pool(name="vpool", bufs=NSC))
        xt_pool = ctx.enter_context(tc.tile_pool(name="xt", bufs=8))
        vtmp_pool = ctx.enter_context(tc.tile_pool(name="vtmp", bufs=2))
        sexp_pool = ctx.enter_context(tc.tile_pool(name="sexp", bufs=6))
        y_pool = ctx.enter_context(tc.tile_pool(name="ypool", bufs=4))
        recip_pool = ctx.enter_context(tc.tile_pool(name="recip", bufs=3))
        rb_pool = ctx.enter_context(tc.tile_pool(name="rb", bufs=3))
        outs_pool = ctx.enter_context(tc.tile_pool(name="outs", bufs=4))

        # ---- constants ----
        wq_sb = consts.tile([128, C // 128, F], ATT)
        wk_sb = consts.tile([128, C // 128, F], ATT)
        wv_sb = consts.tile([128, C // 128, F], ATT)
        nc.sync.dma_start(out=wq_sb, in_=wqT[:].rearrange("(c p) f -> p c f", p=128))
        nc.sync.dma_start(out=wk_sb, in_=wkT[:].rearrange("(c p) f -> p c f", p=128))
        nc.sync.dma_start(out=wv_sb, in_=wvT[:].rearrange("(c p) f -> p c f", p=128))
        wp_sb = consts.tile([F, C], F32R)
        nc.sync.dma_start(out=wp_sb, in_=wpT[:])
        bq_sb = consts.tile([F, 1], FP32)
        bk_sb = consts.tile([F, 1], FP32)
        nc.sync.dma_start(out=bq_sb, in_=bq[:])
        nc.sync.dma_start(out=bk_sb, in_=bk[:])
        # bv broadcast to all 128 partitions (t rows) via stride-0 DMA
        bv_b = consts.tile([128, F], FP32)
        bv_ap = bv[:]
        bv_bcast = bass.AP(
            tensor=bv_ap.tensor, offset=bv_ap.offset,
            ap=[[0, 128]] + [list(d) for d in bv_ap.ap[1:]],
        )
        nc.sync.dma_start(out=bv_b, in_=bv_bcast)
        mask_sb = consts.tile([SC, SC], ATT)
        nc.sync.dma_start(out=mask_sb, in_=mask[:])
        ident_sb = consts.tile([128, 128], FP32)
        nc.sync.dma_start(out=ident_sb, in_=ident[:])

        # ---- persistent activations ----
        qT_sb = persist.tile([F, T], QK)   # (2h x 64 feats, tokens)
        kT_sb = persist.tile([F, T], QK)
        # v chunks: natural (s-token, d) layout + ones column per head
        v_sb = [vpool.tile([SC, 2, D + 1], ATT, tag="v_sb", name=f"v_sb{i}") for i in range(NSC)]
        ones_c = consts.tile([SC, 2, 1], FP32)
        nc.vector.memset(ones_c, 1.0)
        for sc in range(NSC):
            nc.vector.tensor_copy(v_sb[sc][:, :, D:D + 1], ones_c[:])

        # ================= Phase 1: QKV projections =================
        with tc.tile_pool(name="pqkv", bufs=6, space="PSUM") as pqkv, \
                tc.tile_pool(name="pvn", bufs=2, space="PSUM") as pvn_pool:
            for tb in range(NTB):
                t0 = tb * TB
                pq = pqkv.tile([128, TB], FP32, tag="pqkv")
                pk = pqkv.tile([128, TB], FP32, tag="pqkv")
                pv = pqkv.tile([128, TB], FP32, tag="pqkv")
                for c in range(C // 128):
                    xt = xt_pool.tile([128, TB], ATT)
                    nc.sync.dma_start(
                        out=xt, in_=xT[c * 128:(c + 1) * 128, t0:t0 + TB])
                    st = (c == 0)
                    sp = (c == C // 128 - 1)
                    nc.tensor.matmul(pq[:], wq_sb[:, c, :], xt[:],
                                     start=st, stop=sp)
                    nc.tensor.matmul(pk[:], wk_sb[:, c, :], xt[:],
                                     start=st, stop=sp)
                    nc.tensor.matmul(pv[:], wv_sb[:, c, :], xt[:],
                                     start=st, stop=sp)
                # q/k: psum -> sbuf with fused per-partition bias add
                nc.vector.tensor_scalar_add(qT_sb[:, t0:t0 + TB], pq[:], bq_sb[:])
                nc.vector.tensor_scalar_add(kT_sb[:, t0:t0 + TB], pk[:], bk_sb[:])
                # v: psum (f, t) -> sbuf tmp -> PE transpose -> (t, f) + bias
                vt = vtmp_pool.tile([128, TB], FP32)
                nc.vector.tensor_copy(vt[:], pv[:])
                for ts in range(TB // SC):
                    pvn = pvn_pool.tile([128, 128], FP32)
                    nc.tensor.transpose(pvn[:], vt[:, ts * SC:(ts + 1) * SC],
                                        ident_sb[:])
                    sc_idx = tb * (TB // SC) + ts
                    for h in range(2):
                        nc.vector.tensor_add(
                            v_sb[sc_idx][:, h, 0:D],
                            pvn[:, h * D:(h + 1) * D],
                            bv_b[:, h * D:(h + 1) * D])

        # ============ Phase 2+3: attention + output projection ============
        GRP = 2  # score chunks per wide PSUM tile / ACTIVATE
        with tc.tile_pool(name="ps", bufs=2, space="PSUM") as ps_pool, \
                tc.tile_pool(name="py", bufs=2, space="PSUM") as py_pool, \
                tc.tile_pool(name="pp", bufs=2, space="PSUM") as pp_pool:
            for qt in range(NQT):
                q0 = qt * QT
                y_qt = y_pool.tile([F, QT], F32R)  # (2h x 64, q) normalized
                for h in range(2):
                    hs = slice(h * D, (h + 1) * D)
                    py = py_pool.tile([D + 1, QT], FP32)
                    n_sc = (qt + 1) * (QT // SC)
                    chunks = []
                    for sc in range(n_sc):
                        q_off = max(0, sc * SC - q0)
                        chunks.append((sc, q_off, QT - q_off))
                    for g0 in range(0, n_sc, GRP):
                        grp = chunks[g0:g0 + GRP]
                        # scores: (128 s, width q) = k_chunk @ q^T per chunk,
                        # bank-aligned slots in one wide PSUM tile
                        ps = ps_pool.tile([128, GRP * QT], FP32, tag="ps")
                        for j, (sc, q_off, width) in enumerate(grp):
                            nc.tensor.matmul(
                                ps[:, j * QT:j * QT + width],
                                kT_sb[hs, sc * SC:(sc + 1) * SC],
                                qT_sb[hs, q0 + q_off:q0 + QT],
                                start=True, stop=True)
        # one wide exp over the whole group (scale fused) when the
                        # group is gap-free; per-slot exps otherwise
                        sexp = sexp_pool.tile([128, GRP * QT], ATT, tag="sexp")
                        if all(w == QT for (_, _, w) in grp):
                            nc.scalar.activation(
                                sexp[:, :len(grp) * QT], ps[:, :len(grp) * QT],
                                mybir.ActivationFunctionType.Exp,
                                scale=SOFTMAX_SCALE)
                        else:
                            for j, (sc, q_off, width) in enumerate(grp):
                                nc.scalar.activation(
                                    sexp[:, j * QT:j * QT + width],
                                    ps[:, j * QT:j * QT + width],
                                    mybir.ActivationFunctionType.Exp,
                                    scale=SOFTMAX_SCALE)
                        # diagonal chunks: mask the leading triangle block
                        for j, (sc, q_off, width) in enumerate(grp):
                            if sc * SC >= q0:
                                nc.vector.tensor_mul(
                                    sexp[:, j * QT:j * QT + SC],
                                    sexp[:, j * QT:j * QT + SC], mask_sb[:])
                        # AV accumulate: [v | 1] gives y'^T rows + sums row
                        for j, (sc, q_off, width) in enumerate(grp):
                            nc.tensor.matmul(
                                py[:, q_off:QT],
                                v_sb[sc][:, h, :],
                                sexp[:, j * QT:j * QT + width],
                                start=(sc == 0), stop=(sc == n_sc - 1))
                    # normalize: y^T = y'^T * (1/sums) broadcast over d rows
                    sums = recip_pool.tile([1, QT], FP32, tag="sums")
                    nc.vector.tensor_copy(sums[:], py[D:D + 1, :])
                    recip = recip_pool.tile([1, QT], FP32, tag="recip")
                    nc.vector.reciprocal_approx_fast(recip[:], sums[:])
                    rb = rb_pool.tile([D, QT], FP32)
                    nc.gpsimd.partition_broadcast(rb[:], recip[:])
                    nc.vector.tensor_mul(y_qt[hs, :], py[0:D, :], rb[:])
                # output projection for this token block (K=128, both heads)
                for ts in range(QT // 128):
                    tt = q0 + ts * 128
                    for jc in range(C // TB):
                        pp = pp_pool.tile([128, TB], FP32)
                        nc.tensor.matmul(
                            pp[:],
                            y_qt[:, ts * 128:(ts + 1) * 128],
                            wp_sb[:, jc * TB:(jc + 1) * TB],
                            start=True, stop=True)
                        ob = outs_pool.tile([128, TB], FP32)
                        nc.vector.tensor_copy(ob[:], pp[:])
                        nc.sync.dma_start(
                            out=out_p[tt:tt + 128, jc * TB:(jc + 1) * TB],
                            in_=ob[:])


def get_nc():
    global _nc_cache
    if _nc_cache is None:
        nc = bacc.Bacc(None, target_bir_lowering=False)
        _build(nc)
        nc.compile()
        _nc_cache = nc
    return _nc_cache


def _att_cast(a):
    if PRECISION == "bf16":
        import ml_dtypes
        return np.asarray(a, dtype=ml_dtypes.bfloat16)
    return round_fp32r(a)


def make_in_maps(x, Wq, bq, Wk, bk, Wv, bv, Wp):
    x = np.asarray(x, dtype=np.float32)
    Wq, bq = np.asarray(Wq, np.float32), np.asarray(bq, np.float32)
    Wk, bk = np.asarray(Wk, np.float32), np.asarray(bk, np.float32)
    Wv, bv = np.asarray(Wv, np.float32), np.asarray(bv, np.float32)
    Wp = np.asarray(Wp, np.float32)
    xT = _att_cast(np.ascontiguousarray(x.reshape(T, C).T))
    tri = np.tril(np.ones((SC, SC), np.float32)).T  # mask[s,q] = 1 if s<=q
    eye = np.eye(128, dtype=np.float32)
    in_maps = []
    for c in range(N_CORES):
        rows = slice(2 * c * D, 2 * c * D + F)
        in_maps.append({
            "xT": xT,
            "wqT": _att_cast(np.ascontiguousarray(Wq[rows].T)),
            "wkT": _att_cast(np.ascontiguousarray(Wk[rows].T)),
            "wvT": _att_cast(np.ascontiguousarray(Wv[rows].T)),
            "bq": np.ascontiguousarray(bq[rows].reshape(F, 1)),
            "bk": np.ascontiguousarray(bk[rows].reshape(F, 1)),
            "bv": np.ascontiguousarray(bv[rows].reshape(1, F)),
            "wpT": round_fp32r(Wp[:, rows].T),
            "mask": _att_cast(tri),
            "ident": eye,
        })
    return in_maps


def kernel(x, Wq, bq, Wk, bk, Wv, bv, Wp, bp):
    global last_results
    in_maps = make_in_maps(x, Wq, bq, Wk, bk, Wv, bv, Wp)
    nc = get_nc()
    kw = dict(TRACE_KW)
    if TRACE:
        kw["trace"] = True
    res = run_bass_kernel_spmd(nc, in_maps, core_ids=list(range(N_CORES)), **kw)
    last_results = res
    out = res.results[0]["out_p"].astype(np.float64)
    for r in res.results[1:]:
        out += r["out_p"]
    out += np.asarray(bp, np.float64)[None, :]
    return out.reshape(1, T, C).astype(np.float32)


# revision 18
# speedup vs baseline: 1.0456x; 1.0456x over previous
"""Causal self-attention (B=1, T=4096, C=1024, H=16) on 8 TRN2 NeuronCores.

Sharding: head/tensor parallel. Core c owns heads (2c, 2c+1):
  - Wq/Wk/Wv column-split by head -> each core computes its 128 q/k/v features.
  - Attention is local per head (no cross-core comm).
  - Wp row-split: each core computes a partial (T, C) output; host sums the 8
    partials and adds bp.

Device layouts (per core):
  xT   (C=1024, T=4096)  x transposed (replicated), contraction dim on partitions
  qT/kT (128, T)         2 heads stacked on partitions (64 each)
  v    (T-chunks, 65)    natural layout + ones column for softmax row-sums
  scores computed transposed: S^T[s, q] = k @ q^T per head, so
    - exp on ScalarE (scale=1/sqrt(D) fused)
    - sums via the v ones-column during the AV matmul
    - AV output y^T (d, q) is directly the stationary operand for the Wp matmul
"""

import numpy as np

import concourse.bass as bass
import concourse.bacc as bacc
import concourse.mybir as mybir
import concourse.tile as tile
from concourse.bass_utils import run_bass_kernel_spmd

T = 4096
C = 1024
H = 16
D = 64
N_CORES = 8
F = 128          # features per core (2 heads x 64)
TB = 512         # token block for QKV/proj phases
NTB = T // TB    # 8
SC = 128         # key/s chunk
NSC = T // SC    # 32
QT = 512         # q tile in attention phase
NQT = T // QT    # 8

FP32 = mybir.dt.float32
F32R = mybir.dt.float32r
BF16 = mybir.dt.bfloat16

# "f32r"  : all matmuls fp32r (tightest: ~1.6e-4 rel err, but fp32r's 4-byte
#           xbus streams intermittently trip a 50% PE duty-cycle cap -> 304-423us)
# "mix"   : q/k stored bf16 (scores matmul bf16, value path fp32r): ~4.4e-4
#           rel err, stable ~298us. Default.
# "bf16"  : whole attention path bf16 (~2.3e-3 rel err, ~306us)
PRECISION = "mix"

SOFTMAX_SCALE = 1.0 / float(np.sqrt(D))

# Set by test harness to enable NTFF profiling; results stashed in last_results.
TRACE = False
TRACE_KW = {}
last_results = None

_nc_cache = None


def round_fp32r(a):
    """Round fp32 array to the fp32r grid (RNE at 12 low mantissa bits),
    matching the PE's fp32r operand format."""
    u = np.ascontiguousarray(a, np.float32).view(np.uint32).astype(np.uint64)
    low = u & 0xFFF
    base = u & 0xFFFFF000
    rnd = (low > 0x800) | ((low == 0x800) & (((u >> 12) & 1) == 1))
    base = base + rnd.astype(np.uint64) * 0x1000
    return (base & 0xFFFFFFFF).astype(np.uint32).view(np.float32)


def _build(nc):
    ATT = BF16 if PRECISION == "bf16" else F32R
    xT = nc.dram_tensor("xT", [C, T], ATT, kind="ExternalInput")
    wqT = nc.dram_tensor("wqT", [C, F], ATT, kind="ExternalInput")
    wkT = nc.dram_tensor("wkT", [C, F], ATT, kind="ExternalInput")
    wvT = nc.dram_tensor("wvT", [C, F], ATT, kind="ExternalInput")
    bq = nc.dram_tensor("bq", [F, 1], FP32, kind="ExternalInput")
    bk = nc.dram_tensor("bk", [F, 1], FP32, kind="ExternalInput")
    bv = nc.dram_tensor("bv", [1, F], FP32, kind="ExternalInput")
    wpT = nc.dram_tensor("wpT", [F, C], F32R, kind="ExternalInput")
    mask = nc.dram_tensor("mask", [SC, SC], ATT, kind="ExternalInput")
    ident = nc.dram_tensor("ident", [128, 128], FP32, kind="ExternalInput")
    out_p = nc.dram_tensor("out_p", [T, C], FP32, kind="ExternalOutput")

    with tile.TileContext(nc) as tc:
        _body(tc, nc, xT, wqT, wkT, wvT, bq, bk, bv, wpT, mask, ident, out_p)
    return nc


def _body(tc, nc, xT, wqT, wkT, wvT, bq, bk, bv, wpT, mask, ident, out_p):
    ATT = BF16 if PRECISION == "bf16" else F32R
    QK = BF16 if PRECISION in ("bf16", "mix") else F32R
    from contextlib import ExitStack
    with ExitStack() as ctx:
        consts = ctx.enter_context(tc.tile_pool(name="consts", bufs=1))
        persist = ctx.enter_context(tc.tile_pool(name="persist", bufs=1))
        vpool = ctx.enter_context(tc.tile_pool(name="vpool", bufs=NSC))
        xt_pool = ctx.enter_context(tc.tile_pool(name="xt", bufs=8))
        vtmp_pool = ctx.enter_context(tc.tile_pool(name="vtmp", bufs=2))
        sexp_pool = ctx.enter_context(tc.tile_pool(name="sexp", bufs=6))
        y_pool = ctx.enter_context(tc.tile_pool(name="ypool", bufs=4))
        recip_pool = ctx.enter_context(tc.tile_pool(name="recip", bufs=3))
        rb_pool = ctx.enter_context(tc.tile_pool(name="rb", bufs=3))
        outs_pool = ctx.enter_context(tc.tile_pool(name="outs", bufs=4))

        # ---- constants ----
        wq_sb = consts.tile([128, C // 128, F], ATT)
        wk_sb = consts.tile([128, C // 128, F], ATT)
        wv_sb = consts.tile([128, C // 128, F], ATT)
        nc.sync.dma_start(out=wq_sb, in_=wqT[:].rearrange("(c p) f -> p c f", p=128))
        nc.sync.dma_start(out=wk_sb, in_=wkT[:].rearrange("(c p) f -> p c f", p=128))
        nc.sync.dma_start(out=wv_sb, in_=wvT[:].rearrange("(c p) f -> p c f", p=128))
        # late-needed constants: tiles declared here, DMAs emitted inside
        # phase 1 (after the first xt loads) so the PE start is not
        # serialized behind them on the SP DMA queue
        wp_sb = consts.tile([F, C], F32R)
        bq_sb = consts.tile([F, 1], FP32)
        bk_sb = consts.tile([F, 1], FP32)
        bv_b = consts.tile([128, F], FP32)
        mask_sb = consts.tile([SC, SC], ATT)
        ident_sb = consts.tile([128, 128], FP32)

        def _late_const_dmas():
            nc.sync.dma_start(out=bq_sb, in_=bq[:])
            nc.sync.dma_start(out=bk_sb, in_=bk[:])
            bv_ap = bv[:]
            bv_bcast = bass.AP(
                tensor=bv_ap.tensor, offset=bv_ap.offset,
                ap=[[0, 128]] + [list(d) for d in bv_ap.ap[1:]],
            )
            nc.sync.dma_start(out=bv_b, in_=bv_bcast)
            nc.sync.dma_start(out=ident_sb, in_=ident[:])
            nc.sync.dma_start(out=mask_sb, in_=mask[:])
            nc.sync.dma_start(out=wp_sb, in_=wpT[:])

        # ---- persistent activations ----
        qT_sb = persist.tile([F, T], QK)   # (2h x 64 feats, tokens)
        kT_sb = persist.tile([F, T], QK)
        # v chunks: natural (s-token, d) layout + ones column per head
        v_sb = [vpool.tile([SC, 2, D + 1], ATT, tag="v_sb", name=f"v_sb{i}") for i in range(NSC)]
        ones_c = consts.tile([SC, 2, 1], FP32)
        nc.vector.memset(ones_c, 1.0)
        for sc in range(NSC):
            nc.vector.tensor_copy(v_sb[sc][:, :, D:D + 1], ones_c[:])

        # ================= Phase 1: QKV projections =================
        with tc.tile_pool(name="pqkv", bufs=6, space="PSUM") as pqkv, \
                tc.tile_pool(name="pvn", bufs=2, space="PSUM") as pvn_pool:
            for tb in range(NTB):
                t0 = tb * TB
                pq = pqkv.tile([128, TB], FP32, tag="pqkv")
                pk = pqkv.tile([128, TB], FP32, tag="pqkv")
                pv = pqkv.tile([128, TB], FP32, tag="pqkv")
                for c in range(C // 128):
                    xt = xt_pool.tile([128, TB], ATT)
                    nc.sync.dma_start(
                        out=xt, in_=xT[c * 128:(c + 1) * 128, t0:t0 + TB])
                    st = (c == 0)
                    sp = (c == C // 128 - 1)
                    nc.tensor.matmul(pq[:], wq_sb[:, c, :], xt[:],
                                     start=st, stop=sp)
                    nc.tensor.matmul(pk[:], wk_sb[:, c, :], xt[:],
                                     start=st, stop=sp)
                    nc.tensor.matmul(pv[:], wv_sb[:, c, :], xt[:],
                                     start=st, stop=sp)
                if tb == 0:
                    _late_const_dmas()
                # q/k: psum -> sbuf with fused per-partition bias add
                nc.vector.tensor_scalar_add(qT_sb[:, t0:t0 + TB], pq[:], bq_sb[:])
                nc.vector.tensor_scalar_add(kT_sb[:, t0:t0 + TB], pk[:], bk_sb[:])
                # v: psum (f, t) -> sbuf tmp -> PE transpose -> (t, f) + bias
                vt = vtmp_pool.tile([128, TB], FP32)
                nc.vector.tensor_copy(vt[:], pv[:])
                for ts in range(TB // SC):
                    pvn = pvn_pool.tile([128, 128], FP32)
                    nc.tensor.transpose(pvn[:], vt[:, ts * SC:(ts + 1) * SC],
                                        ident_sb[:])
                    sc_idx = tb * (TB // SC) + ts
                    for h in range(2):
                        nc.vector.tensor_add(
                            v_sb[sc_idx][:, h, 0:D],
                            pvn[:, h * D:(h + 1) * D],
                            bv_b[:, h * D:(h + 1) * D])

        # ============ Phase 2+3: attention + output projection ============
        GRP = 2  # score chunks per wide PSUM tile / ACTIVATE
        with tc.tile_pool(name="ps", bufs=2, space="PSUM") as ps_pool, \
                tc.tile_pool(name="py", bufs=2, space="PSUM") as py_pool, \
                tc.tile_pool(name="pp", bufs=2, space="PSUM") as pp_pool:
            for qt in range(NQT):
                q0 = qt * QT
                y_qt = y_pool.tile([F, QT], F32R)  # (2h x 64, q) normalized
                for h in range(2):
                    hs = slice(h * D, (h + 1) * D)
                    py = py_pool.tile([D + 1, QT], FP32)
                    n_sc = (qt + 1) * (QT // SC)
                    chunks = []
                    for sc in range(n_sc):
                        q_off = max(0, sc * SC - q0)
                        chunks.append((sc, q_off, QT - q_off))
                    for g0 in range(0, n_sc, GRP):
                        grp = chunks[g0:g0 + GRP]
                        # scores: (128 s, width q) = k_chunk @ q^T per chunk,
                        # bank-aligned slots in one wide PSUM tile
                        ps = ps_pool.tile([128, GRP * QT], FP32, tag="ps")
                        for j, (sc, q_off, width) in enumerate(grp):
                            nc.tensor.matmul(
                                ps[:, j * QT:j * QT + width],
                                kT_sb[hs, sc * SC:(sc + 1) * SC],
                                qT_sb[hs, q0 + q_off:q0 + QT],
                                start=True, stop=True)
        # one wide exp over the whole group (scale fused) when the
                        # group is gap-free; per-slot exps otherwise
                        sexp = sexp_pool.tile([128, GRP * QT], ATT, tag="sexp")
                        if all(w == QT for (_, _, w) in grp):
                            nc.scalar.activation(
                                sexp[:, :len(grp) * QT], ps[:, :len(grp) * QT],
                                mybir.ActivationFunctionType.Exp,
                                scale=SOFTMAX_SCALE)
                        else:
                            for j, (sc, q_off, width) in enumerate(grp):
                                nc.scalar.activation(
                                    sexp[:, j * QT:j * QT + width],
                                    ps[:, j * QT:j * QT + width],
                                    mybir.ActivationFunctionType.Exp,
                                    scale=SOFTMAX_SCALE)
                        # diagonal chunks: mask the leading triangle block
                        for j, (sc, q_off, width) in enumerate(grp):
                            if sc * SC >= q0:
                                nc.vector.tensor_mul(
                                    sexp[:, j * QT:j * QT + SC],
                                    sexp[:, j * QT:j * QT + SC], mask_sb[:])
                        # AV accumulate: [v | 1] gives y'^T rows + sums row
                        for j, (sc, q_off, width) in enumerate(grp):
                            nc.tensor.matmul(
                                py[:, q_off:QT],
                                v_sb[sc][:, h, :],
                                sexp[:, j * QT:j * QT + width],
                                start=(sc == 0), stop=(sc == n_sc - 1))
                    # normalize: y^T = y'^T * (1/sums) broadcast over d rows
                    sums = recip_pool.tile([1, QT], FP32, tag="sums")
                    nc.vector.tensor_copy(sums[:], py[D:D + 1, :])
                    recip = recip_pool.tile([1, QT], FP32, tag="recip")
                    nc.vector.reciprocal_approx_fast(recip[:], sums[:])
                    rb = rb_pool.tile([D, QT], FP32)
                    nc.gpsimd.partition_broadcast(rb[:], recip[:])
                    nc.vector.tensor_mul(y_qt[hs, :], py[0:D, :], rb[:])
                # output projection for this token block (K=128, both heads)
                for ts in range(QT // 128):
                    tt = q0 + ts * 128
                    for jc in range(C // TB):
                        pp = pp_pool.tile([128, TB], FP32)
                        nc.tensor.matmul(
                            pp[:],
                            y_qt[:, ts * 128:(ts + 1) * 128],
                            wp_sb[:, jc * TB:(jc + 1) * TB],
                            start=True, stop=True)
                        ob = outs_pool.tile([128, TB], FP32)
                        nc.vector.tensor_copy(ob[:], pp[:])
                        nc.sync.dma_start(
                            out=out_p[tt:tt + 128, jc * TB:(jc + 1) * TB],
                            in_=ob[:])


def get_nc():
    global _nc_cache
    if _nc_cache is None:
        nc = bacc.Bacc(None, target_bir_lowering=False)
        _build(nc)
        nc.compile()
        _nc_cache = nc
    return _nc_cache


def _att_cast(a):
    if PRECISION == "bf16":
        import ml_dtypes
        return np.asarray(a, dtype=ml_dtypes.bfloat16)
    return round_fp32r(a)


def make_in_maps(x, Wq, bq, Wk, bk, Wv, bv, Wp):
    x = np.asarray(x, dtype=np.float32)
    Wq, bq = np.asarray(Wq, np.float32), np.asarray(bq, np.float32)
    Wk, bk = np.asarray(Wk, np.float32), np.asarray(bk, np.float32)
    Wv, bv = np.asarray(Wv, np.float32), np.asarray(bv, np.float32)
    Wp = np.asarray(Wp, np.float32)
    xT = _att_cast(np.ascontiguousarray(x.reshape(T, C).T))
    tri = np.tril(np.ones((SC, SC), np.float32)).T  # mask[s,q] = 1 if s<=q
    eye = np.eye(128, dtype=np.float32)
    in_maps = []
    for c in range(N_CORES):
        rows = slice(2 * c * D, 2 * c * D + F)
        in_maps.append({
            "xT": xT,
            "wqT": _att_cast(np.ascontiguousarray(Wq[rows].T)),
            "wkT": _att_cast(np.ascontiguousarray(Wk[rows].T)),
            "wvT": _att_cast(np.ascontiguousarray(Wv[rows].T)),
            "bq": np.ascontiguousarray(bq[rows].reshape(F, 1)),
            "bk": np.ascontiguousarray(bk[rows].reshape(F, 1)),
            "bv": np.ascontiguousarray(bv[rows].reshape(1, F)),
            "wpT": round_fp32r(Wp[:, rows].T),
            "mask": _att_cast(tri),
            "ident": eye,
        })
    return in_maps


def kernel(x, Wq, bq, Wk, bk, Wv, bv, Wp, bp):
    global last_results
    in_maps = make_in_maps(x, Wq, bq, Wk, bk, Wv, bv, Wp)
    nc = get_nc()
    kw = dict(TRACE_KW)
    if TRACE:
        kw["trace"] = True
    res = run_bass_kernel_spmd(nc, in_maps, core_ids=list(range(N_CORES)), **kw)
    last_results = res
    out = res.results[0]["out_p"].astype(np.float64)
    for r in res.results[1:]:
        out += r["out_p"]
    out += np.asarray(bp, np.float64)[None, :]
    return out.reshape(1, T, C).astype(np.float32)


# revision 19
# speedup vs baseline: 1.0515x; 1.0056x over previous
"""Causal self-attention (B=1, T=4096, C=1024, H=16) on 8 TRN2 NeuronCores.

Sharding: head/tensor parallel. Core c owns heads (2c, 2c+1):
  - Wq/Wk/Wv column-split by head -> each core computes its 128 q/k/v features.
  - Attention is local per head (no cross-core comm).
  - Wp row-split: each core computes a partial (T, C) output; host sums the 8
    partials and adds bp.

Device layouts (per core):
  xT   (C=1024, T=4096)  x transposed (replicated), contraction dim on partitions
  qT/kT (128, T)         2 heads stacked on partitions (64 each)
  v    (T-chunks, 65)    natural layout + ones column for softmax row-sums
  scores computed transposed: S^T[s, q] = k @ q^T per head, so
    - exp on ScalarE (scale=1/sqrt(D) fused)
    - sums via the v ones-column during the AV matmul
    - AV output y^T (d, q) is directly the stationary operand for the Wp matmul
"""

import numpy as np

import concourse.bass as bass
import concourse.bacc as bacc
import concourse.mybir as mybir
import concourse.tile as tile
from concourse.bass_utils import run_bass_kernel_spmd

T = 4096
C = 1024
H = 16
D = 64
N_CORES = 8
F = 128          # features per core (2 heads x 64)
TB = 512         # token block for QKV/proj phases
NTB = T // TB    # 8
SC = 128         # key/s chunk
NSC = T // SC    # 32
QT = 512         # q tile in attention phase
NQT = T // QT    # 8

FP32 = mybir.dt.float32
F32R = mybir.dt.float32r
BF16 = mybir.dt.bfloat16

# "f32r"  : all matmuls fp32r (tightest: ~1.6e-4 rel err, but fp32r's 4-byte
#           xbus streams intermittently trip a 50% PE duty-cycle cap -> 304-423us)
# "mix"   : q/k stored bf16 (scores matmul bf16, value path fp32r): ~4.4e-4
#           rel err, stable ~298us. Default.
# "bf16"  : whole attention path bf16 (~2.3e-3 rel err, ~306us)
PRECISION = "mix"

SOFTMAX_SCALE = 1.0 / float(np.sqrt(D))

# Set by test harness to enable NTFF profiling; results stashed in last_results.
TRACE = False
TRACE_KW = {}
last_results = None

_nc_cache = None


def round_fp32r(a):
    """Round fp32 array to the fp32r grid (RNE at 12 low mantissa bits),
    matching the PE's fp32r operand format."""
    u = np.ascontiguousarray(a, np.float32).view(np.uint32).astype(np.uint64)
    low = u & 0xFFF
    base = u & 0xFFFFF000
    rnd = (low > 0x800) | ((low == 0x800) & (((u >> 12) & 1) == 1))
    base = base + rnd.astype(np.uint64) * 0x1000
    return (base & 0xFFFFFFFF).astype(np.uint32).view(np.float32)


def _build(nc):
    ATT = BF16 if PRECISION == "bf16" else F32R
    xT = nc.dram_tensor("xT", [C, T], ATT, kind="ExternalInput")
    wqT = nc.dram_tensor("wqT", [C, F], ATT, kind="ExternalInput")
    wkT = nc.dram_tensor("wkT", [C, F], ATT, kind="ExternalInput")
    wvT = nc.dram_tensor("wvT", [C, F], ATT, kind="ExternalInput")
    bq = nc.dram_tensor("bq", [F, 1], FP32, kind="ExternalInput")
    bk = nc.dram_tensor("bk", [F, 1], FP32, kind="ExternalInput")
    bv = nc.dram_tensor("bv", [1, F], FP32, kind="ExternalInput")
    wpT = nc.dram_tensor("wpT", [F, C], F32R, kind="ExternalInput")
    mask = nc.dram_tensor("mask", [SC, SC], ATT, kind="ExternalInput")
    ident = nc.dram_tensor("ident", [128, 128], FP32, kind="ExternalInput")
    out_p = nc.dram_tensor("out_p", [T, C], FP32, kind="ExternalOutput")

    with tile.TileContext(nc) as tc:
        _body(tc, nc, xT, wqT, wkT, wvT, bq, bk, bv, wpT, mask, ident, out_p)
    return nc


def _body(tc, nc, xT, wqT, wkT, wvT, bq, bk, bv, wpT, mask, ident, out_p):
    ATT = BF16 if PRECISION == "bf16" else F32R
    QK = BF16 if PRECISION in ("bf16", "mix") else F32R
    from contextlib import ExitStack
    with ExitStack() as ctx:
        consts = ctx.enter_context(tc.tile_pool(name="consts", bufs=1))
        persist = ctx.enter_context(tc.tile_pool(name="persist", bufs=1))
        vpool = ctx.enter_context(tc.tile_pool(name="vpool", bufs=NSC))
        xt_pool = ctx.enter_context(tc.tile_pool(name="xt", bufs=8))
        vtmp_pool = ctx.enter_context(tc.tile_pool(name="vtmp", bufs=2))
        sexp_pool = ctx.enter_context(tc.tile_pool(name="sexp", bufs=8))
        y_pool = ctx.enter_context(tc.tile_pool(name="ypool", bufs=4))
        recip_pool = ctx.enter_context(tc.tile_pool(name="recip", bufs=3))
        rb_pool = ctx.enter_context(tc.tile_pool(name="rb", bufs=3))
        outs_pool = ctx.enter_context(tc.tile_pool(name="outs", bufs=4))

        # ---- constants ----
        wq_sb = consts.tile([128, C // 128, F], ATT)
        wk_sb = consts.tile([128, C // 128, F], ATT)
        wv_sb = consts.tile([128, C // 128, F], ATT)
        nc.sync.dma_start(out=wq_sb, in_=wqT[:].rearrange("(c p) f -> p c f", p=128))
        nc.sync.dma_start(out=wk_sb, in_=wkT[:].rearrange("(c p) f -> p c f", p=128))
        nc.sync.dma_start(out=wv_sb, in_=wvT[:].rearrange("(c p) f -> p c f", p=128))
        # late-needed constants: tiles declared here, DMAs emitted inside
        # phase 1 (after the first xt loads) so the PE start is not
        # serialized behind them on the SP DMA queue
        wp_sb = consts.tile([F, C], F32R)
        bq_sb = consts.tile([F, 1], FP32)
        bk_sb = consts.tile([F, 1], FP32)
        bv_b = consts.tile([128, F], FP32)
        mask_sb = consts.tile([SC, SC], ATT)
        ident_sb = consts.tile([128, 128], FP32)

        def _late_const_dmas():
            nc.sync.dma_start(out=bq_sb, in_=bq[:])
            nc.sync.dma_start(out=bk_sb, in_=bk[:])
            bv_ap = bv[:]
            bv_bcast = bass.AP(
                tensor=bv_ap.tensor, offset=bv_ap.offset,
                ap=[[0, 128]] + [list(d) for d in bv_ap.ap[1:]],
            )
            nc.sync.dma_start(out=bv_b, in_=bv_bcast)
            nc.sync.dma_start(out=ident_sb, in_=ident[:])
            nc.sync.dma_start(out=mask_sb, in_=mask[:])
            nc.sync.dma_start(out=wp_sb, in_=wpT[:])

        # ---- persistent activations ----
        qT_sb = persist.tile([F, T], QK)   # (2h x 64 feats, tokens)
        kT_sb = persist.tile([F, T], QK)
        # v chunks: natural (s-token, d) layout + ones column per head
        v_sb = [vpool.tile([SC, 2, D + 1], ATT, tag="v_sb", name=f"v_sb{i}") for i in range(NSC)]
        ones_c = consts.tile([SC, 2, 1], FP32)
        nc.vector.memset(ones_c, 1.0)
        # dummy exp: pulls the ~2.7us ACT exp-table load into the QKV phase
        # (ScalarE is idle there) instead of stalling the first real exp
        dummy = consts.tile([1, 1], FP32)
        nc.scalar.activation(dummy[:], ones_c[0:1, 0, 0:1],
                             mybir.ActivationFunctionType.Exp)
        for sc in range(NSC):
            nc.vector.tensor_copy(v_sb[sc][:, :, D:D + 1], ones_c[:])

        # ================= Phase 1: QKV projections =================
        with tc.tile_pool(name="pqkv", bufs=6, space="PSUM") as pqkv, \
                tc.tile_pool(name="pvn", bufs=2, space="PSUM") as pvn_pool:
            for tb in range(NTB):
                t0 = tb * TB
                pq = pqkv.tile([128, TB], FP32, tag="pqkv")
                pk = pqkv.tile([128, TB], FP32, tag="pqkv")
                pv = pqkv.tile([128, TB], FP32, tag="pqkv")
                for c in range(C // 128):
                    xt = xt_pool.tile([128, TB], ATT)
                    nc.sync.dma_start(
                        out=xt, in_=xT[c * 128:(c + 1) * 128, t0:t0 + TB])
                    st = (c == 0)
                    sp = (c == C // 128 - 1)
                    nc.tensor.matmul(pq[:], wq_sb[:, c, :], xt[:],
                                     start=st, stop=sp)
                    nc.tensor.matmul(pk[:], wk_sb[:, c, :], xt[:],
                                     start=st, stop=sp)
                    nc.tensor.matmul(pv[:], wv_sb[:, c, :], xt[:],
                                     start=st, stop=sp)
                if tb == 0:
                    _late_const_dmas()
                # q/k: psum -> sbuf with fused per-partition bias add
                nc.vector.tensor_scalar_add(qT_sb[:, t0:t0 + TB], pq[:], bq_sb[:])
                nc.vector.tensor_scalar_add(kT_sb[:, t0:t0 + TB], pk[:], bk_sb[:])
                # v: psum (f, t) -> sbuf tmp -> PE transpose -> (t, f) + bias
                vt = vtmp_pool.tile([128, TB], FP32)
                nc.vector.tensor_copy(vt[:], pv[:])
                for ts in range(TB // SC):
                    pvn = pvn_pool.tile([128, 128], FP32)
                    nc.tensor.transpose(pvn[:], vt[:, ts * SC:(ts + 1) * SC],
                                        ident_sb[:])
                    sc_idx = tb * (TB // SC) + ts
                    for h in range(2):
                        nc.vector.tensor_add(
                            v_sb[sc_idx][:, h, 0:D],
                            pvn[:, h * D:(h + 1) * D],
                            bv_b[:, h * D:(h + 1) * D])

        # ============ Phase 2+3: attention + output projection ============
        GRP = 2  # score chunks per wide PSUM tile / ACTIVATE
        with tc.tile_pool(name="ps", bufs=2, space="PSUM") as ps_pool, \
                tc.tile_pool(name="py", bufs=2, space="PSUM") as py_pool, \
                tc.tile_pool(name="pp", bufs=2, space="PSUM") as pp_pool:
            for qt in range(NQT):
                q0 = qt * QT
                y_qt = y_pool.tile([F, QT], F32R)  # (2h x 64, q) normalized
                for h in range(2):
                    hs = slice(h * D, (h + 1) * D)
                    py = py_pool.tile([D + 1, QT], FP32)
                    n_sc = (qt + 1) * (QT // SC)
                    chunks = []
                    for sc in range(n_sc):
                        q_off = max(0, sc * SC - q0)
                        chunks.append((sc, q_off, QT - q_off))
                    for g0 in range(0, n_sc, GRP):
                        grp = chunks[g0:g0 + GRP]
                        # scores: (128 s, width q) = k_chunk @ q^T per chunk,
                        # bank-aligned slots in one wide PSUM tile
                        ps = ps_pool.tile([128, GRP * QT], FP32, tag="ps")
                        for j, (sc, q_off, width) in enumerate(grp):
                            nc.tensor.matmul(
                                ps[:, j * QT:j * QT + width],
                                kT_sb[hs, sc * SC:(sc + 1) * SC],
                                qT_sb[hs, q0 + q_off:q0 + QT],
                                start=True, stop=True)
        # one wide exp over the whole group (scale fused) when the
                        # group is gap-free; per-slot exps otherwise
                        sexp = sexp_pool.tile([128, GRP * QT], ATT, tag="sexp")
                        if all(w == QT for (_, _, w) in grp):
                            nc.scalar.activation(
                                sexp[:, :len(grp) * QT], ps[:, :len(grp) * QT],
                                mybir.ActivationFunctionType.Exp,
                                scale=SOFTMAX_SCALE)
                        else:
                            for j, (sc, q_off, width) in enumerate(grp):
                                nc.scalar.activation(
                                    sexp[:, j * QT:j * QT + width],
                                    ps[:, j * QT:j * QT + width],
                                    mybir.ActivationFunctionType.Exp,
                                    scale=SOFTMAX_SCALE)
                        # diagonal chunks: mask the leading triangle block
                        for j, (sc, q_off, width) in enumerate(grp):
                            if sc * SC >= q0:
                                nc.vector.tensor_mul(
                                    sexp[:, j * QT:j * QT + SC],
                                    sexp[:, j * QT:j * QT + SC], mask_sb[:])
                        # AV accumulate: [v | 1] gives y'^T rows + sums row
                        for j, (sc, q_off, width) in enumerate(grp):
                            nc.tensor.matmul(
                                py[:, q_off:QT],
                                v_sb[sc][:, h, :],
                                sexp[:, j * QT:j * QT + width],
                                start=(sc == 0), stop=(sc == n_sc - 1))
                    # normalize: y^T = y'^T * (1/sums) broadcast over d rows
                    sums = recip_pool.tile([1, QT], FP32, tag="sums")
                    nc.vector.tensor_copy(sums[:], py[D:D + 1, :])
                    recip = recip_pool.tile([1, QT], FP32, tag="recip")
                    nc.vector.reciprocal_approx_fast(recip[:], sums[:])
                    rb = rb_pool.tile([D, QT], FP32)
                    nc.gpsimd.partition_broadcast(rb[:], recip[:])
                    nc.vector.tensor_mul(y_qt[hs, :], py[0:D, :], rb[:])
                # output projection for this token block (K=128, both heads)
                for ts in range(QT // 128):
                    tt = q0 + ts * 128
                    for jc in range(C // TB):
                        pp = pp_pool.tile([128, TB], FP32)
                        nc.tensor.matmul(
                            pp[:],
                            y_qt[:, ts * 128:(ts + 1) * 128],
                            wp_sb[:, jc * TB:(jc + 1) * TB],
                            start=True, stop=True)
                        ob = outs_pool.tile([128, TB], FP32)
                        nc.vector.tensor_copy(ob[:], pp[:])
                        nc.sync.dma_start(
                            out=out_p[tt:tt + 128, jc * TB:(jc + 1) * TB],
                            in_=ob[:])


def get_nc():
    global _nc_cache
    if _nc_cache is None:
        nc = bacc.Bacc(None, target_bir_lowering=False)
        _build(nc)
        nc.compile()
        _nc_cache = nc
    return _nc_cache


def _att_cast(a):
    if PRECISION == "bf16":
        import ml_dtypes
        return np.asarray(a, dtype=ml_dtypes.bfloat16)
    return round_fp32r(a)


def make_in_maps(x, Wq, bq, Wk, bk, Wv, bv, Wp):
    x = np.asarray(x, dtype=np.float32)
    Wq, bq = np.asarray(Wq, np.float32), np.asarray(bq, np.float32)
    Wk, bk = np.asarray(Wk, np.float32), np.asarray(bk, np.float32)
    Wv, bv = np.asarray(Wv, np.float32), np.asarray(bv, np.float32)
    Wp = np.asarray(Wp, np.float32)
    xT = _att_cast(np.ascontiguousarray(x.reshape(T, C).T))
    tri = np.tril(np.ones((SC, SC), np.float32)).T  # mask[s,q] = 1 if s<=q
    eye = np.eye(128, dtype=np.float32)
    in_maps = []
    for c in range(N_CORES):
        rows = slice(2 * c * D, 2 * c * D + F)
        in_maps.append({
            "xT": xT,
            "wqT": _att_cast(np.ascontiguousarray(Wq[rows].T)),
            "wkT": _att_cast(np.ascontiguousarray(Wk[rows].T)),
            "wvT": _att_cast(np.ascontiguousarray(Wv[rows].T)),
            "bq": np.ascontiguousarray(bq[rows].reshape(F, 1)),
            "bk": np.ascontiguousarray(bk[rows].reshape(F, 1)),
            "bv": np.ascontiguousarray(bv[rows].reshape(1, F)),
            "wpT": round_fp32r(Wp[:, rows].T),
            "mask": _att_cast(tri),
            "ident": eye,
        })
    return in_maps


def kernel(x, Wq, bq, Wk, bk, Wv, bv, Wp, bp):
    global last_results
    in_maps = make_in_maps(x, Wq, bq, Wk, bk, Wv, bv, Wp)
    nc = get_nc()
    kw = dict(TRACE_KW)
    if TRACE:
        kw["trace"] = True
    res = run_bass_kernel_spmd(nc, in_maps, core_ids=list(range(N_CORES)), **kw)
    last_results = res
    out = res.results[0]["out_p"].astype(np.float64)
    for r in res.results[1:]:
        out += r["out_p"]
    out += np.asarray(bp, np.float64)[None, :]
    return out.reshape(1, T, C).astype(np.float32)
